# revision 1
# baseline (speedup 1.0000x reference)
"""ConvMod kernel for Trainium2 (8 NeuronCores, batch-parallel).

Per-sample modulated 3x3 grouped conv:
  style = w @ (fce_kernel*fce_scale) + fce_bias                [B, CIN]
  wp    = conv_kernel * conv_scale * style                     [B,3,3,CIN,NF]
  wpp   = wp * rsqrt(sum(wp^2, (ky,kx,cin)) + 1e-8)            demodulated
  out   = conv2d_same(x, wpp per-sample) + conv_bias           [B,H,W,NF]

Sharding: batch B=8 across 8 cores (1 sample/core), params replicated.

Device layout: M=128 matmul packing. PSUM partitions hold (2 output rows x
64 channels). The x tile duplicates channels on partitions 64-127 shifted
by +2 rows, so one K=128 matmul contracts two input rows at once with a
block-structured lhsT. Per 4 output rows (supergroup): 6 matmuls of
free-size 512 cover all 9 taps for all 4 rows. Output is written fp16
(tolerance 2e-2 >> fp16 rounding) to halve store DMA traffic.
"""

import numpy as np

B, H, W, CIN = 8, 256, 256, 64
WDIM, NF, KK = 512, 64, 3
NCORES = 8
CR = 32                 # output rows per x chunk
NCH = H // CR           # 8 chunks
SGC = CR // 4           # supergroups (4 output rows) per chunk
FCE_SCALE = float(np.sqrt(1.0 / WDIM))
CONV_SCALE = float(np.sqrt(1.0 / 0.6 / (KK * KK * CIN)))

_CACHE = {}


def _build(repeats=1):
    import concourse.bass as bass_mod
    import concourse.mybir as mybir
    import concourse.tile as tile
    from concourse import bacc

    f32 = mybir.dt.float32
    f32r = mybir.dt.float32r
    f16 = mybir.dt.float16
    nc = bacc.Bacc("TRN2", target_bir_lowering=False, debug=False,
                   num_devices=NCORES)

    xt = nc.dram_tensor("xt", [CIN, H, W], f32r, kind="ExternalInput").ap()
    wv = nc.dram_tensor("wv", [WDIM], f32, kind="ExternalInput").ap()
    fce_k = nc.dram_tensor("fce_k", [WDIM, CIN], f32, kind="ExternalInput").ap()
    fce_b = nc.dram_tensor("fce_b", [CIN], f32, kind="ExternalInput").ap()
    ck_d = nc.dram_tensor("ck", [KK, KK, CIN, NF], f32, kind="ExternalInput").ap()
    cb_d = nc.dram_tensor("cb", [NF], f32, kind="ExternalInput").ap()
    # out: partition p = ro*64 + n (ro = row parity), free = (g, col) with
    # output row = 2g + ro
    ytd = nc.dram_tensor("ytd", [2 * NF, (H // 2) * W], f16,
                         kind="ExternalOutput").ap()

    NT = KK * KK  # 9 taps
    with tile.TileContext(nc) as tc:
        with (
            tc.tile_pool(name="const", bufs=1) as const,
            tc.tile_pool(name="prep", bufs=1) as prep,
            tc.tile_pool(name="pps", bufs=1, space="PSUM") as pps,
            tc.tile_pool(name="xin", bufs=1) as xin,
            tc.tile_pool(name="yout", bufs=4) as yout,
            tc.tile_pool(name="acc", bufs=7, space="PSUM") as accp,
        ):
            # ---- weight prep (tiny) ----
            # dummy Sqrt issued first: hoists the 2.5us activation-table
            # load off the demod critical chain (runs on the idle ACT
            # engine while DMAs stream)
            dmy = const.tile([1, 1], f32)
            nc.vector.memset(dmy, 1.0)
            dmy2 = prep.tile([1, 1], f32)
            nc.scalar.sqrt(dmy2, dmy)
            # weight loads on SP ahead of the x segments: they gate the
            # whole demod chain
            fce_sb = prep.tile([128, WDIM // 128, CIN], f32)
            nc.sync.dma_start(out=fce_sb,
                              in_=fce_k.rearrange("(j p) c -> p j c", p=128))
            wv_sb = prep.tile([128, WDIM // 128], f32)
            nc.sync.dma_start(out=wv_sb,
                              in_=wv.rearrange("(j p) -> p j", p=128))
            ck_sb = prep.tile([CIN, NT, NF], f32)
            nc.sync.dma_start(out=ck_sb,
                              in_=ck_d.rearrange("ky kx c n -> c ky kx n"))
            fce_b_sb = prep.tile([CIN, 1], f32)
            nc.sync.dma_start(out=fce_b_sb, in_=fce_b)
            # conv_bias on both row-parity halves of the output partitions
            cb2_sb = const.tile([2 * NF, 1], f32)
            nc.scalar.dma_start(out=cb2_sb[0:NF, :], in_=cb_d)
            nc.scalar.dma_start(out=cb2_sb[NF:2 * NF, :], in_=cb_d)

            # f32r zero row for all padding writes (memset cannot emit
            # f32r); built first -- depends on nothing, keeps the border
            # writes off the first-matmul critical path
            zf258 = prep.tile([CIN, W + 2], f32)
            nc.vector.memset(zf258, 0.0)
            zrow = const.tile([CIN, 1, W + 2], f32r)
            nc.vector.tensor_mul(zrow.rearrange("c a w -> c (a w)"),
                                 zf258, zf258)

            # persistent x tiles: A half = x rows R0-1..R0+CR (CR+2),
            # B half (partitions 64-127) = A shifted +2 rows. Col 0 and
            # W+1 of the A half are zero borders (B inherits via the copy).
            xxb = [xin.tile([2 * CIN, CR + 2, W + 2], f32r, name=f"xx{k}")
                   for k in range(4)]
            zcol = zrow[:, 0:1, 0:CR + 2].rearrange("c a w -> c w a")
            for k in range(4):
                nc.vector.tensor_copy(xxb[k][0:CIN, :, 0:1], zcol)
                nc.vector.tensor_copy(xxb[k][0:CIN, :, W + 1:W + 2], zcol)

            ones_k = const.tile([CIN, 1], f32)
            nc.vector.memset(ones_k, 1.0)
            ones_m = const.tile([1, NF], f32)
            nc.vector.memset(ones_m, 1.0)

            # style = w @ (fce_k * fce_scale) + fce_b, then * conv_scale
            # (all prep matmul outputs packed into one PSUM bank; each is
            # consumed before the next accumulation group begins)
            ppsbig = pps.tile([128, 512], f32)
            style_ps = ppsbig[0:CIN, 0:1]
            for j in range(WDIM // 128):
                nc.tensor.matmul(style_ps, lhsT=fce_sb[:, j, :],
                                 rhs=wv_sb[:, j:j + 1],
                                 start=(j == 0), stop=(j == WDIM // 128 - 1))
            fce_b_sc = prep.tile([CIN, 1], f32)
            nc.scalar.mul(out=fce_b_sc, in_=fce_b_sb, mul=CONV_SCALE)
            stylec = prep.tile([CIN, 1], f32)
            nc.scalar.activation(stylec, style_ps,
                                 mybir.ActivationFunctionType.Identity,
                                 bias=fce_b_sc, scale=FCE_SCALE * CONV_SCALE)

            # wp[c, t, n] = ck * stylec[c];  sq = wp^2
            wp = prep.tile([CIN, NT, NF], f32)
            nc.vector.tensor_scalar_mul(wp.rearrange("c t n -> c (t n)"),
                                        ck_sb.rearrange("c t n -> c (t n)"),
                                        stylec)
            sq = prep.tile([CIN, NT, NF], f32)
            nc.vector.tensor_mul(sq.rearrange("c t n -> c (t n)"),
                                 wp.rearrange("c t n -> c (t n)"),
                                 wp.rearrange("c t n -> c (t n)"))
            t4 = prep.tile([CIN, 4, NF], f32)
            nc.vector.tensor_add(t4.rearrange("c t n -> c (t n)"),
                                 sq[:, 0:4, :].rearrange("c t n -> c (t n)"),
                                 sq[:, 4:8, :].rearrange("c t n -> c (t n)"))
            t2 = prep.tile([CIN, 2, NF], f32)
            nc.vector.tensor_add(t2.rearrange("c t n -> c (t n)"),
                                 t4[:, 0:2, :].rearrange("c t n -> c (t n)"),
                                 t4[:, 2:4, :].rearrange("c t n -> c (t n)"))
            tap_acc = prep.tile([CIN, NF], f32)
            nc.vector.tensor_add(tap_acc, t2[:, 0, :], t2[:, 1, :])
            nc.vector.tensor_add(tap_acc, tap_acc, sq[:, 8, :])
            ssum_ps = ppsbig[0:1, 64:64 + NF]
            nc.tensor.matmul(ssum_ps, lhsT=ones_k, rhs=tap_acc,
                             start=True, stop=True)
            eps_sb = prep.tile([1, 1], f32)
            nc.vector.memset(eps_sb, 1e-8)
            sroot = prep.tile([1, NF], f32)
            nc.scalar.activation(sroot, ssum_ps,
                                 mybir.ActivationFunctionType.Sqrt,
                                 bias=eps_sb, scale=1.0)
            wstd = prep.tile([1, NF], f32)
            nc.vector.reciprocal(wstd, sroot)
            bcast_ps = ppsbig[0:CIN, 128:128 + NF]
            nc.tensor.matmul(bcast_ps, lhsT=ones_m, rhs=wstd,
                             start=True, stop=True)
            # Block-structured lhsT tiles for the M=128 scheme, demod mul
            # fused in: L block = wp[:, t, :] * bcast_ps (-> f32r).
            # lhsT[k, m]: k<64 = channels of x row XA, k>=64 = x row XA+2;
            # m<64 = out row r (ro=0) channels, m>=64 = out row r+1 (ro=1).
            # mm1 (XA = r-1): (A,ro0)=w[-1,s-1] (B,ro0)=w[+1,s-1]
            #                 (B,ro1)=w[0,s-1]  (A,ro1)=0
            # mm2 (XA = r):   (A,ro0)=w[0,s-1]  (A,ro1)=w[-1,s-1]
            #                 (B,ro1)=w[+1,s-1] (B,ro0)=0
            # wp tap index t = (dy+1)*3 + (dx+1), dx = s-1.
            zf3 = prep.tile([CIN, KK, NF], f32)
            nc.vector.memset(zf3.rearrange("c s n -> c (s n)"), 0.0)
            L1 = const.tile([2 * CIN, KK, 2 * NF], f32r)
            L2 = const.tile([2 * CIN, KK, 2 * NF], f32r)
            mul = nc.vector.tensor_mul
            bc3 = bcast_ps.unsqueeze(1).broadcast_to([CIN, KK, NF])
            mul(L1[0:CIN, :, 0:NF], wp[:, 0:3, :], bc3)
            mul(L1[CIN:2 * CIN, :, 0:NF], wp[:, 6:9, :], bc3)
            mul(L1[CIN:2 * CIN, :, NF:2 * NF], wp[:, 3:6, :], bc3)
            mul(L1[0:CIN, :, NF:2 * NF], zf3, zf3)
            mul(L2[0:CIN, :, 0:NF], wp[:, 3:6, :], bc3)
            mul(L2[0:CIN, :, NF:2 * NF], wp[:, 0:3, :], bc3)
            mul(L2[CIN:2 * CIN, :, NF:2 * NF], wp[:, 6:9, :], bc3)
            mul(L2[CIN:2 * CIN, :, 0:NF], zf3, zf3)



            # ---- main conv loop (software-pipelined emission) ----
            # Chunk ci+1's segment loads and dup bands are emitted inside
            # chunk ci's supergroup loop so every engine queue interleaves
            # producer work for the next chunk with consumer work for the
            # current one. PSUM->staging drain alternates ACT/DVE so
            # neither engine paces PE.
            # chunks 0-1: fine granularity (4 segs/bands) to shrink the
            # pipeline fill; later chunks: coarse (2 segs/bands) to halve
            # the sync-instruction count on the critical path
            SEGS_F = [0, 10, 18, 26, CR + 2]
            SEGS_C = [0, 18, CR + 2]
            BANDS_F = [0, 8, 16, 24, 32]
            BANDS_C = [0, 16, 32]

            def nseg(cj):
                return 4 if cj < 2 else 2

            def emit_loads(cj):
                l0 = 1 if cj == 0 else 2
                l1 = CR + 1 if cj == NCH - 1 else CR + 2
                R0 = cj * CR
                segs = SEGS_F if cj < 2 else SEGS_C
                for si in range(nseg(cj)):
                    a = max(segs[si], l0)
                    b = min(segs[si + 1], l1)
                    nc.sync.dma_start(
                        out=xxb[cj % 4][0:CIN, a:b, 1:W + 1],
                        in_=xt[:, R0 - 1 + a:R0 - 1 + b, :])

            def emit_band(cj, si):
                xx = xxb[cj % 4]
                if si == 0:
                    if cj == 0:
                        nc.vector.tensor_copy(xx[0:CIN, 0:1, :], zrow)
                    else:
                        nc.vector.tensor_copy(
                            xx[0:CIN, 0:2, :],
                            xxb[(cj - 1) % 4][0:CIN, CR:CR + 2, :])
                bands = BANDS_F if cj < 2 else BANDS_C
                if si == nseg(cj) - 1 and cj == NCH - 1:
                    nc.vector.tensor_copy(xx[0:CIN, CR + 1:CR + 2, :], zrow)
                ba, bb = bands[si], bands[si + 1]
                eng = nc.gpsimd if (cj < 2 and si == 2) else nc.vector
                eng.tensor_copy(xx[CIN:2 * CIN, ba:bb, :],
                                xx[0:CIN, ba + 2:bb + 2, :])

            for _ in range(repeats):
                emit_loads(0)
                emit_loads(1)
                for si in range(4):
                    emit_band(0, si)
                for ci in range(NCH):
                    xx = xxb[ci % 4]
                    xxr = xx.rearrange("p (a b) w -> p b a w", b=2)
                    for q in range(SGC):
                        if q == 0 and ci + 2 < NCH:
                            emit_loads(ci + 2)
                        if ci + 1 < NCH and q % 2 == 1:
                            si = (q - 1) // 2
                            if si < nseg(ci + 1):
                                emit_band(ci + 1, si)
                        sg = ci * SGC + q
                        k8 = sg % 8
                        if k8 == 0:
                            ys = yout.tile([2 * NF, 8 * 2 * W], f16)
                        ps = accp.tile([2 * NF, 2 * W], f32)
                        for s in range(KK):
                            nc.tensor.matmul(
                                ps, lhsT=L1[:, s, :],
                                rhs=xxr[:, 0, 2 * q:2 * q + 2, s:s + W],
                                start=(s == 0), stop=False)
                        for s in range(KK):
                            nc.tensor.matmul(
                                ps, lhsT=L2[:, s, :],
                                rhs=xxr[:, 1, 2 * q:2 * q + 2, s:s + W],
                                start=False, stop=(s == KK - 1))
                        yslice = ys[:, k8 * 2 * W:(k8 + 1) * 2 * W]
                        if q % 2 == 0:
                            nc.scalar.activation(
                                yslice, ps,
                                mybir.ActivationFunctionType.Identity,
                                bias=cb2_sb, scale=1.0)
                        else:
                            nc.vector.tensor_scalar_add(yslice, ps, cb2_sb)
                        if k8 == 7:
                            nc.scalar.dma_start(
                                out=ytd[:, (sg - 7) * 2 * W:(sg + 1) * 2 * W],
                                in_=ys)

    nc.compile()
    return nc


def _get(repeats=1):
    if repeats not in _CACHE:
        _CACHE[repeats] = _build(repeats)
    return _CACHE[repeats]


def kernel(x, w, fce_kernel, fce_bias, conv_kernel, conv_bias):
    from concourse.bass_utils import run_bass_kernel_spmd

    nc = _get()
    in_maps = []
    for b in range(B):
        in_maps.append({
            "xt": np.ascontiguousarray(np.asarray(x[b], np.float32).transpose(2, 0, 1)),
            "wv": np.ascontiguousarray(np.asarray(w[b], np.float32)),
            "fce_k": np.asarray(fce_kernel, np.float32),
            "fce_b": np.asarray(fce_bias, np.float32),
            "ck": np.asarray(conv_kernel, np.float32),
            "cb": np.asarray(conv_bias, np.float32),
        })
    res = run_bass_kernel_spmd(nc, in_maps, core_ids=list(range(NCORES)))
    out = np.empty((B, H, W, NF), np.float32)
    for b in range(B):
        a = np.asarray(res.results[b]["ytd"]).astype(np.float32)
        # [ro*64+n, g*W+col] -> [h, w, n] with h = 2g + ro
        a = a.reshape(2, NF, H // 2, W).transpose(2, 0, 3, 1)
        out[b] = a.reshape(H, W, NF)
    return out



# revision 4
# speedup vs baseline: 1.0539x; 1.0539x over previous
"""ConvMod kernel for Trainium2 (8 NeuronCores, batch-parallel).

Per-sample modulated 3x3 grouped conv:
  style = w @ (fce_kernel*fce_scale) + fce_bias                [B, CIN]
  wp    = conv_kernel * conv_scale * style                     [B,3,3,CIN,NF]
  wpp   = wp * rsqrt(sum(wp^2, (ky,kx,cin)) + 1e-8)            demodulated
  out   = conv2d_same(x, wpp per-sample) + conv_bias           [B,H,W,NF]

Sharding: batch B=8 across 8 cores (1 sample/core), params replicated.

Device layout: M=128 matmul packing. PSUM partitions hold (2 output rows x
64 channels). The x tile duplicates channels on partitions 64-127 shifted
by +2 rows, so one K=128 matmul contracts two input rows at once with a
block-structured lhsT. Per 4 output rows (supergroup): 6 matmuls of
free-size 512 cover all 9 taps for all 4 rows.

Since demod scales only the output channel, conv(x, wp*diag(wstd)) =
conv(x, wp)*wstd[n]: the conv runs with un-demodulated weights and wstd
is applied as a per-partition scale in the PSUM->SBUF drain (fused with
the conv_bias add). This takes the whole sqrt/rsqrt chain off the
critical path to the first matmul. ssum[n] = sum_c stylec[c]^2 *
(sum_t ck[c,t,n]^2) collapses the demod reduction to one matmul with a
style-independent ckk tensor, landing directly as [64,1] partitions.

x and conv weights travel as fp16 (tolerance 2e-2 >> fp16 rounding):
halves input DMA traffic and on-chip dup-copy cost; matmul rate is
identical for f32r/f16 on TRN2. Output written fp16 for the same reason.
"""

import numpy as np

B, H, W, CIN = 8, 256, 256, 64
WDIM, NF, KK = 512, 64, 3
NCORES = 8
CR = 32                 # output rows per x chunk
NCH = H // CR           # 8 chunks
SGC = CR // 4           # supergroups (4 output rows) per chunk
FCE_SCALE = float(np.sqrt(1.0 / WDIM))
CONV_SCALE = float(np.sqrt(1.0 / 0.6 / (KK * KK * CIN)))

_CACHE = {}


def _build(repeats=1):
    import concourse.bass as bass_mod
    import concourse.mybir as mybir
    import concourse.tile as tile
    from concourse import bacc

    f32 = mybir.dt.float32
    f16 = mybir.dt.float16
    nc = bacc.Bacc("TRN2", target_bir_lowering=False, debug=False,
                   num_devices=NCORES)

    xt = nc.dram_tensor("xt", [CIN, H, W], f16, kind="ExternalInput").ap()
    wv = nc.dram_tensor("wv", [WDIM], f32, kind="ExternalInput").ap()
    fce_k = nc.dram_tensor("fce_k", [WDIM, CIN], f32, kind="ExternalInput").ap()
    fce_b = nc.dram_tensor("fce_b", [CIN], f32, kind="ExternalInput").ap()
    ck_d = nc.dram_tensor("ck", [KK, KK, CIN, NF], f16, kind="ExternalInput").ap()
    cb_d = nc.dram_tensor("cb", [NF], f32, kind="ExternalInput").ap()
    # out: partition p = ro*64 + n (ro = row parity), free = (g, col) with
    # output row = 2g + ro
    ytd = nc.dram_tensor("ytd", [2 * NF, (H // 2) * W], f16,
                         kind="ExternalOutput").ap()

    NT = KK * KK  # 9 taps
    with tile.TileContext(nc) as tc:
        with (
            tc.tile_pool(name="const", bufs=1) as const,
            tc.tile_pool(name="prep", bufs=1) as prep,
            tc.tile_pool(name="pps", bufs=1, space="PSUM") as pps,
            tc.tile_pool(name="xin", bufs=1) as xin,
            tc.tile_pool(name="yout", bufs=4) as yout,
            tc.tile_pool(name="acc", bufs=7, space="PSUM") as accp,
        ):
            # dummy Sqrt issued first: hoists the activation-table load off
            # the critical chain (runs on ACT while DMAs stream)
            dmy = const.tile([1, 1], f32)
            nc.vector.memset(dmy, 1.0)
            dmy2 = prep.tile([1, 1], f32)
            nc.scalar.sqrt(dmy2, dmy)

            # SP/HWDGE queue: fce_k + wv ahead of the x segments (they gate
            # the style chain); everything else rides the Pool/SWDGE path
            # so HWDGE reaches the x loads sooner.
            fce_sb = prep.tile([128, WDIM // 128, CIN], f32)
            nc.sync.dma_start(out=fce_sb,
                              in_=fce_k.rearrange("(j p) c -> p j c", p=128))
            wv_sb = prep.tile([128, WDIM // 128], f32)
            nc.sync.dma_start(out=wv_sb,
                              in_=wv.rearrange("(j p) -> p j", p=128))
            # Pool/SWDGE: fce_b first (gates stylec), then ck (gates L),
            # then conv_bias halves (needed only at the first drain)
            fce_b_sb = prep.tile([CIN, 1], f32)
            nc.gpsimd.dma_start(out=fce_b_sb, in_=fce_b)
            ck_sb = prep.tile([CIN, NT, NF], f16)
            nc.gpsimd.dma_start(out=ck_sb,
                                in_=ck_d.rearrange("ky kx c n -> c ky kx n"))
            cb2_sb = const.tile([2 * NF, 1], f32)
            nc.gpsimd.dma_start(out=cb2_sb[0:NF, :], in_=cb_d)
            nc.gpsimd.dma_start(out=cb2_sb[NF:2 * NF, :], in_=cb_d)

            # zero row for padding writes (f16 memset is fine)
            zrow = const.tile([CIN, 1, W + 2], f16)
            nc.vector.memset(zrow.rearrange("c a w -> c (a w)"), 0.0)

            # persistent x tiles: A half = x rows R0-1..R0+CR (CR+2),
            # B half (partitions 64-127) = A shifted +2 rows. Col 0 and
            # W+1 of the A half are zero borders (B inherits via the copy).
            xxb = [xin.tile([2 * CIN, CR + 2, W + 2], f16, name=f"xx{k}")
                   for k in range(4)]
            zcol = zrow[:, 0:1, 0:CR + 2].rearrange("c a w -> c w a")
            for k in range(4):
                nc.vector.tensor_copy(xxb[k][0:CIN, :, 0:1], zcol)
                nc.vector.tensor_copy(xxb[k][0:CIN, :, W + 1:W + 2], zcol)

            # style = (w @ fce_k)*fce_scale*conv_scale + fce_b*conv_scale
            ppsbig = pps.tile([128, 512], f32)
            style_ps = ppsbig[0:CIN, 0:1]
            for j in range(WDIM // 128):
                nc.tensor.matmul(style_ps, lhsT=fce_sb[:, j, :],
                                 rhs=wv_sb[:, j:j + 1],
                                 start=(j == 0), stop=(j == WDIM // 128 - 1))
            fce_b_sc = prep.tile([CIN, 1], f32)
            nc.scalar.mul(out=fce_b_sc, in_=fce_b_sb, mul=CONV_SCALE)
            stylec = prep.tile([CIN, 1], f32)
            nc.scalar.activation(stylec, style_ps,
                                 mybir.ActivationFunctionType.Identity,
                                 bias=fce_b_sc, scale=FCE_SCALE * CONV_SCALE)

            # Block-structured lhsT tiles for the M=128 scheme, straight
            # from ck * stylec (no demod mul -- applied at drain time).
            # lhsT[k, m]: k<64 = channels of x row XA, k>=64 = x row XA+2;
            # m<64 = out row r (ro=0) channels, m>=64 = out row r+1 (ro=1).
            # mm1 (XA = r-1): (A,ro0)=w[-1,s-1] (B,ro0)=w[+1,s-1]
            #                 (B,ro1)=w[0,s-1]  (A,ro1)=0
            # mm2 (XA = r):   (A,ro0)=w[0,s-1]  (A,ro1)=w[-1,s-1]
            #                 (B,ro1)=w[+1,s-1] (B,ro0)=0
            # wp tap index t = (dy+1)*3 + (dx+1), dx = s-1.
            L1 = const.tile([2 * CIN, KK, 2 * NF], f16)
            L2 = const.tile([2 * CIN, KK, 2 * NF], f16)
            # zero quadrants: Pool memsets, emitted first on that engine
            # (no deps) -- but after its DMA gens in program order, so put
            # them here where Pool is otherwise idle
            nc.gpsimd.memset(L1[0:CIN, :, NF:2 * NF], 0.0)
            nc.gpsimd.memset(L2[CIN:2 * CIN, :, 0:NF], 0.0)

            def lmul(eng, dst, t0):
                src = ck_sb[:, t0:t0 + 3, :]
                if eng == "v":
                    nc.vector.tensor_scalar_mul(dst, src, stylec)
                else:
                    nc.scalar.mul(out=dst, in_=src, mul=stylec)

            # L1 quadrants on DVE (gates mm1-3), L2 on ACT (gates mm4-6)
            lmul("v", L1[0:CIN, :, 0:NF], 0)
            lmul("v", L1[CIN:2 * CIN, :, 0:NF], 6)
            lmul("v", L1[CIN:2 * CIN, :, NF:2 * NF], 3)
            lmul("a", L2[0:CIN, :, 0:NF], 3)
            lmul("a", L2[0:CIN, :, NF:2 * NF], 0)
            lmul("a", L2[CIN:2 * CIN, :, NF:2 * NF], 6)

            # demod chain (off critical path): ckk[c,n] = sum_t ck^2 on
            # Pool; ssum[n] = ckk^T @ stylec^2 lands as [64,1] partitions
            sq = prep.tile([CIN, NT, NF], f32)
            nc.gpsimd.tensor_mul(sq.rearrange("c t n -> c (t n)"),
                                 ck_sb.rearrange("c t n -> c (t n)"),
                                 ck_sb.rearrange("c t n -> c (t n)"))
            t4 = prep.tile([CIN, 4, NF], f32)
            nc.gpsimd.tensor_add(t4.rearrange("c t n -> c (t n)"),
                                 sq[:, 0:4, :].rearrange("c t n -> c (t n)"),
                                 sq[:, 4:8, :].rearrange("c t n -> c (t n)"))
            t2 = prep.tile([CIN, 2, NF], f32)
            nc.gpsimd.tensor_add(t2.rearrange("c t n -> c (t n)"),
                                 t4[:, 0:2, :].rearrange("c t n -> c (t n)"),
                                 t4[:, 2:4, :].rearrange("c t n -> c (t n)"))
            ckk = prep.tile([CIN, NF], f32)
            nc.gpsimd.tensor_add(ckk, t2[:, 0, :], t2[:, 1, :])
            nc.gpsimd.tensor_add(ckk, ckk, sq[:, 8, :])
            stylec2 = prep.tile([CIN, 1], f32)
            nc.vector.tensor_mul(stylec2, stylec, stylec)
            ssum_ps = ppsbig[0:CIN, 64:65]
            nc.tensor.matmul(ssum_ps, lhsT=ckk, rhs=stylec2,
                             start=True, stop=True)
            eps_sb = prep.tile([CIN, 1], f32)
            nc.vector.memset(eps_sb, 1e-8)
            sroot = prep.tile([CIN, 1], f32)
            nc.scalar.activation(sroot, ssum_ps,
                                 mybir.ActivationFunctionType.Sqrt,
                                 bias=eps_sb, scale=1.0)
            wstdT = prep.tile([CIN, 1], f32)
            nc.vector.reciprocal(wstdT, sroot)
            wstd2 = const.tile([2 * NF, 1], f32)
            nc.vector.tensor_copy(wstd2[0:NF, :], wstdT)
            nc.vector.tensor_copy(wstd2[NF:2 * NF, :], wstdT)

            # ---- main conv loop (software-pipelined emission) ----
            # Chunk ci+1's segment loads and dup bands are emitted inside
            # chunk ci's supergroup loop so every engine queue interleaves
            # producer work for the next chunk with consumer work for the
            # current one. PSUM->staging drain alternates ACT/DVE so
            # neither engine paces PE.
            # chunks 0-1: fine granularity (4 segs/bands) to shrink the
            # pipeline fill; later chunks: coarse (2 segs/bands) to halve
            # the sync-instruction count on the critical path
            SEGS_F = [0, 10, 18, 26, CR + 2]
            SEGS_C = [0, 18, CR + 2]
            BANDS_F = [0, 8, 16, 24, 32]
            BANDS_C = [0, 16, 32]

            def nseg(cj):
                return 4 if cj < 2 else 2

            def emit_loads(cj):
                l0 = 1 if cj == 0 else 2
                l1 = CR + 1 if cj == NCH - 1 else CR + 2
                R0 = cj * CR
                segs = SEGS_F if cj < 2 else SEGS_C
                for si in range(nseg(cj)):
                    a = max(segs[si], l0)
                    b = min(segs[si + 1], l1)
                    nc.sync.dma_start(
                        out=xxb[cj % 4][0:CIN, a:b, 1:W + 1],
                        in_=xt[:, R0 - 1 + a:R0 - 1 + b, :])

            def emit_band(cj, si):
                xx = xxb[cj % 4]
                if si == 0:
                    if cj == 0:
                        nc.vector.tensor_copy(xx[0:CIN, 0:1, :], zrow)
                    else:
                        nc.vector.tensor_copy(
                            xx[0:CIN, 0:2, :],
                            xxb[(cj - 1) % 4][0:CIN, CR:CR + 2, :])
                bands = BANDS_F if cj < 2 else BANDS_C
                if si == nseg(cj) - 1 and cj == NCH - 1:
                    nc.vector.tensor_copy(xx[0:CIN, CR + 1:CR + 2, :], zrow)
                ba, bb = bands[si], bands[si + 1]
                nc.vector.tensor_copy(xx[CIN:2 * CIN, ba:bb, :],
                                      xx[0:CIN, ba + 2:bb + 2, :])

            for _ in range(repeats):
                emit_loads(0)
                emit_loads(1)
                for si in range(4):
                    emit_band(0, si)
                for ci in range(NCH):
                    xx = xxb[ci % 4]
                    xxr = xx.rearrange("p (a b) w -> p b a w", b=2)
                    for q in range(SGC):
                        if q == 0 and ci + 2 < NCH:
                            emit_loads(ci + 2)
                        if ci + 1 < NCH and q % 2 == 1:
                            si = (q - 1) // 2
                            if si < nseg(ci + 1):
                                emit_band(ci + 1, si)
                        sg = ci * SGC + q
                        k2 = sg % 2
                        if k2 == 0:
                            ys = yout.tile([2 * NF, 2 * 2 * W], f16)
                        ps = accp.tile([2 * NF, 2 * W], f32)
                        for s in range(KK):
                            nc.tensor.matmul(
                                ps, lhsT=L1[:, s, :],
                                rhs=xxr[:, 0, 2 * q:2 * q + 2, s:s + W],
                                start=(s == 0), stop=False)
                        for s in range(KK):
                            nc.tensor.matmul(
                                ps, lhsT=L2[:, s, :],
                                rhs=xxr[:, 1, 2 * q:2 * q + 2, s:s + W],
                                start=False, stop=(s == KK - 1))
                        yslice = ys[:, k2 * 2 * W:(k2 + 1) * 2 * W]
                        # drain applies demod scale + bias:
                        # y = ps*wstd[n] + cb[n]
                        if q % 2 == 0:
                            nc.scalar.activation(
                                yslice, ps,
                                mybir.ActivationFunctionType.Identity,
                                bias=cb2_sb, scale=wstd2)
                        else:
                            nc.vector.tensor_scalar(
                                yslice, ps, wstd2, cb2_sb,
                                op0=mybir.AluOpType.mult,
                                op1=mybir.AluOpType.add)
                        if k2 == 1:
                            nc.scalar.dma_start(
                                out=ytd[:, (sg - 1) * 2 * W:(sg + 1) * 2 * W],
                                in_=ys)

    nc.compile()
    return nc


def _get(repeats=1):
    if repeats not in _CACHE:
        _CACHE[repeats] = _build(repeats)
    return _CACHE[repeats]


def kernel(x, w, fce_kernel, fce_bias, conv_kernel, conv_bias):
    from concourse.bass_utils import run_bass_kernel_spmd

    nc = _get()
    in_maps = []
    for b in range(B):
        in_maps.append({
            "xt": np.ascontiguousarray(
                np.asarray(x[b], np.float32).transpose(2, 0, 1)).astype(np.float16),
            "wv": np.ascontiguousarray(np.asarray(w[b], np.float32)),
            "fce_k": np.asarray(fce_kernel, np.float32),
            "fce_b": np.asarray(fce_bias, np.float32),
            "ck": np.asarray(conv_kernel, np.float32).astype(np.float16),
            "cb": np.asarray(conv_bias, np.float32),
        })
    res = run_bass_kernel_spmd(nc, in_maps, core_ids=list(range(NCORES)))
    out = np.empty((B, H, W, NF), np.float32)
    for b in range(B):
        a = np.asarray(res.results[b]["ytd"]).astype(np.float32)
        # [ro*64+n, g*W+col] -> [h, w, n] with h = 2g + ro
        a = a.reshape(2, NF, H // 2, W).transpose(2, 0, 3, 1)
        out[b] = a.reshape(H, W, NF)
    return out


# revision 7
# speedup vs baseline: 1.1637x; 1.1041x over previous
"""ConvMod kernel for Trainium2 (8 NeuronCores, batch-parallel).

Per-sample modulated 3x3 grouped conv:
  style = w @ (fce_kernel*fce_scale) + fce_bias                [B, CIN]
  wp    = conv_kernel * conv_scale * style                     [B,3,3,CIN,NF]
  wpp   = wp * rsqrt(sum(wp^2, (ky,kx,cin)) + 1e-8)            demodulated
  out   = conv2d_same(x, wpp per-sample) + conv_bias           [B,H,W,NF]

Sharding: batch B=8 across 8 cores (1 sample/core), params replicated.

Device layout: M=128 matmul packing. PSUM partitions hold (2 output rows x
64 channels). The x tile duplicates channels on partitions 64-127 shifted
by +2 rows, so one K=128 matmul contracts two input rows at once with a
block-structured lhsT. Per 4 output rows (supergroup): 6 matmuls of
free-size 512 cover all 9 taps for all 4 rows.

Since demod scales only the output channel, conv(x, wp*diag(wstd)) =
conv(x, wp)*wstd[n]: the conv runs with un-demodulated weights and wstd
is applied as a per-partition scale in the PSUM->SBUF drain (fused with
the conv_bias add). This takes the whole sqrt/rsqrt chain off the
critical path to the first matmul. ssum[n] = sum_c stylec[c]^2 *
(sum_t ck[c,t,n]^2) collapses the demod reduction to one matmul with a
style-independent ckk tensor, landing directly as [64,1] partitions.

x and conv weights travel as fp16 (tolerance 2e-2 >> fp16 rounding):
halves input DMA traffic and on-chip dup-copy cost; matmul rate is
identical for f32r/f16 on TRN2. Output written fp16 for the same reason.
"""

import numpy as np

B, H, W, CIN = 8, 256, 256, 64
WDIM, NF, KK = 512, 64, 3
NCORES = 8
CR = 32                 # output rows per x chunk
NCH = H // CR           # 8 chunks
SGC = CR // 4           # supergroups (4 output rows) per chunk
FCE_SCALE = float(np.sqrt(1.0 / WDIM))
CONV_SCALE = float(np.sqrt(1.0 / 0.6 / (KK * KK * CIN)))

NWARM = 34

_CACHE = {}


def _build(repeats=1):
    import concourse.bass as bass_mod
    import concourse.mybir as mybir
    import concourse.tile as tile
    from concourse import bacc

    f32 = mybir.dt.float32
    f32r = mybir.dt.float32r
    f16 = mybir.dt.float16
    nc = bacc.Bacc("TRN2", target_bir_lowering=False, debug=False,
                   num_devices=NCORES)

    xt = nc.dram_tensor("xt", [CIN, H, W], f16, kind="ExternalInput").ap()
    wv = nc.dram_tensor("wv", [WDIM], f32, kind="ExternalInput").ap()
    fce_k = nc.dram_tensor("fce_k", [WDIM, CIN], f32, kind="ExternalInput").ap()
    fce_b = nc.dram_tensor("fce_b", [CIN], f32, kind="ExternalInput").ap()
    ck_d = nc.dram_tensor("ck", [KK, KK, CIN, NF], f16, kind="ExternalInput").ap()
    cb_d = nc.dram_tensor("cb", [NF], f32, kind="ExternalInput").ap()
    # out: partition p = ro*64 + n (ro = row parity), free = (g, col) with
    # output row = 2g + ro
    ytd = nc.dram_tensor("ytd", [2 * NF, (H // 2) * W], f16,
                         kind="ExternalOutput").ap()

    NT = KK * KK  # 9 taps
    with tile.TileContext(nc) as tc:
        with (
            tc.tile_pool(name="const", bufs=1) as const,
            tc.tile_pool(name="prep", bufs=1) as prep,
            tc.tile_pool(name="pps", bufs=1, space="PSUM") as pps,
            tc.tile_pool(name="xin", bufs=1) as xin,
            tc.tile_pool(name="yout", bufs=4) as yout,
            tc.tile_pool(name="acc", bufs=7, space="PSUM") as accp,
        ):
            # dummy Sqrt issued first: hoists the activation-table load off
            # the critical chain (runs on ACT while DMAs stream)
            dmy = const.tile([1, 1], f32)
            nc.vector.memset(dmy, 1.0)
            dmy2 = prep.tile([1, 1], f32)
            nc.scalar.sqrt(dmy2, dmy)

            # PE warm-up: the cost model's p-state ramp needs ~3us of
            # continuous PE execution to reach full clock, and a long idle
            # gap resets it (costing ~8us of slow matmuls after the fill).
            # Dummy 128-col matmuls keep PE busy from t~0.1us until the
            # style matmuls' inputs land (~3.6us), so the conv runs at
            # 2.4GHz from its very first instruction.
            wrm_l = const.tile([1, 1], f16)
            nc.vector.memset(wrm_l, 0.0)
            wrm_r = const.tile([1, 128], f16)
            nc.vector.memset(wrm_r, 0.0)

            # SP/HWDGE queue: ck first (gates both the L quadrant build
            # and the ckk demod chain), then fce_k + wv (style chain),
            # then the x segments. Small fce_b/cb ride the Pool/SWDGE
            # path, whose descriptor generator runs in parallel.
            ck_sb = prep.tile([CIN, NT, NF], f16)
            nc.sync.dma_start(out=ck_sb,
                              in_=ck_d.rearrange("ky kx c n -> c ky kx n"))
            fce_sb = prep.tile([128, WDIM // 128, CIN], f32)
            nc.sync.dma_start(out=fce_sb,
                              in_=fce_k.rearrange("(j p) c -> p j c", p=128))
            wv_sb = prep.tile([128, WDIM // 128], f32)
            nc.sync.dma_start(out=wv_sb,
                              in_=wv.rearrange("(j p) -> p j", p=128))
            fce_b_sb = prep.tile([CIN, 1], f32)
            nc.gpsimd.dma_start(out=fce_b_sb, in_=fce_b)
            cb2_sb = const.tile([2 * NF, 1], f32)
            nc.gpsimd.dma_start(out=cb2_sb[0:NF, :], in_=cb_d)
            nc.gpsimd.dma_start(out=cb2_sb[NF:2 * NF, :], in_=cb_d)

            # zero row for padding writes (f16 memset is fine)
            zrow = const.tile([CIN, 1, W + 2], f16)
            nc.vector.memset(zrow.rearrange("c a w -> c (a w)"), 0.0)

            # persistent x tiles: A half = x rows R0-1..R0+CR (CR+2),
            # B half (partitions 64-127) = A shifted +2 rows. Col 0 and
            # W+1 of the A half are zero borders (B inherits via the copy).
            xxb = [xin.tile([2 * CIN, CR + 2, W + 2], f16, name=f"xx{k}")
                   for k in range(4)]
            zcol = zrow[:, 0:1, 0:CR + 2].rearrange("c a w -> c w a")
            for k in range(4):
                nc.vector.tensor_copy(xxb[k][0:CIN, :, 0:1], zcol)
                nc.vector.tensor_copy(xxb[k][0:CIN, :, W + 1:W + 2], zcol)

            # style = (w @ fce_k)*fce_scale*conv_scale + fce_b*conv_scale
            ppsbig = pps.tile([128, 512], f32)
            wrm_ps = ppsbig[0:1, 384:512]
            for _w in range(NWARM):
                nc.tensor.matmul(wrm_ps, lhsT=wrm_l, rhs=wrm_r,
                                 start=True, stop=True)
            style_ps = ppsbig[0:CIN, 0:1]
            for j in range(WDIM // 128):
                nc.tensor.matmul(style_ps, lhsT=fce_sb[:, j, :],
                                 rhs=wv_sb[:, j:j + 1],
                                 start=(j == 0), stop=(j == WDIM // 128 - 1))
            fce_b_sc = prep.tile([CIN, 1], f32)
            nc.scalar.mul(out=fce_b_sc, in_=fce_b_sb, mul=CONV_SCALE)
            stylec = prep.tile([CIN, 1], f32)
            nc.scalar.activation(stylec, style_ps,
                                 mybir.ActivationFunctionType.Identity,
                                 bias=fce_b_sc, scale=FCE_SCALE * CONV_SCALE)

            # Block-structured lhsT tiles for the M=128 scheme, straight
            # from ck * stylec (no demod mul -- applied at drain time).
            # lhsT[k, m]: k<64 = channels of x row XA, k>=64 = x row XA+2;
            # m<64 = out row r (ro=0) channels, m>=64 = out row r+1 (ro=1).
            # mm1 (XA = r-1): (A,ro0)=w[-1,s-1] (B,ro0)=w[+1,s-1]
            #                 (B,ro1)=w[0,s-1]  (A,ro1)=0
            # mm2 (XA = r):   (A,ro0)=w[0,s-1]  (A,ro1)=w[-1,s-1]
            #                 (B,ro1)=w[+1,s-1] (B,ro0)=0
            # wp tap index t = (dy+1)*3 + (dx+1), dx = s-1.
            L1 = const.tile([2 * CIN, KK, 2 * NF], f16)
            L2 = const.tile([2 * CIN, KK, 2 * NF], f16)
            nc.gpsimd.memset(L1[0:CIN, :, NF:2 * NF], 0.0)
            nc.gpsimd.memset(L2[CIN:2 * CIN, :, 0:NF], 0.0)

            def lmul(eng, dst, t0):
                src = ck_sb[:, t0:t0 + 3, :]
                if eng == "v":
                    nc.vector.tensor_scalar_mul(dst, src, stylec)
                else:
                    nc.scalar.mul(out=dst, in_=src, mul=stylec)

            # L1 quadrants on DVE (gates mm1-3), L2 on ACT (gates mm4-6)
            lmul("v", L1[0:CIN, :, 0:NF], 0)
            lmul("v", L1[CIN:2 * CIN, :, 0:NF], 6)
            lmul("v", L1[CIN:2 * CIN, :, NF:2 * NF], 3)
            lmul("a", L2[0:CIN, :, 0:NF], 3)
            lmul("a", L2[0:CIN, :, NF:2 * NF], 0)
            lmul("a", L2[CIN:2 * CIN, :, NF:2 * NF], 6)

            # demod chain (off critical path): ckk[c,n] = sum_t ck^2 on
            # Pool; ssum[n] = ckk^T @ stylec^2 lands as [64,1] partitions
            sq = prep.tile([CIN, NT, NF], f32)
            nc.gpsimd.tensor_mul(sq.rearrange("c t n -> c (t n)"),
                                 ck_sb.rearrange("c t n -> c (t n)"),
                                 ck_sb.rearrange("c t n -> c (t n)"))
            t4 = prep.tile([CIN, 4, NF], f32)
            nc.gpsimd.tensor_add(t4.rearrange("c t n -> c (t n)"),
                                 sq[:, 0:4, :].rearrange("c t n -> c (t n)"),
                                 sq[:, 4:8, :].rearrange("c t n -> c (t n)"))
            t2 = prep.tile([CIN, 2, NF], f32)
            nc.gpsimd.tensor_add(t2.rearrange("c t n -> c (t n)"),
                                 t4[:, 0:2, :].rearrange("c t n -> c (t n)"),
                                 t4[:, 2:4, :].rearrange("c t n -> c (t n)"))
            ckk = prep.tile([CIN, NF], f32)
            nc.gpsimd.tensor_add(ckk, t2[:, 0, :], t2[:, 1, :])
            nc.gpsimd.tensor_add(ckk, ckk, sq[:, 8, :])
            stylec2 = prep.tile([CIN, 1], f32)
            nc.vector.tensor_mul(stylec2, stylec, stylec)
            eps_sb = prep.tile([CIN, 1], f32)
            nc.vector.memset(eps_sb, 1e-8)
            ssum_ps = ppsbig[0:CIN, 64:65]
            sroot = prep.tile([CIN, 1], f32)
            wstdT = prep.tile([CIN, 1], f32)
            wstd2 = const.tile([2 * NF, 1], f32)

            def emit_wstd():
                # emitted right after supergroup 0's matmuls: the ssum
                # matmul then sits behind sg0 in the PE queue (ckk is
                # ready by then), and sroot lands on ACT before the first
                # drain, which consumes wstd2.
                nc.tensor.matmul(ssum_ps, lhsT=ckk, rhs=stylec2,
                                 start=True, stop=True)
                nc.scalar.activation(sroot, ssum_ps,
                                     mybir.ActivationFunctionType.Sqrt,
                                     bias=eps_sb, scale=1.0)
                nc.vector.reciprocal(wstdT, sroot)
                nc.vector.tensor_copy(wstd2[0:NF, :], wstdT)
                nc.vector.tensor_copy(wstd2[NF:2 * NF, :], wstdT)

            # ---- main conv loop (software-pipelined emission) ----
            # Chunk ci+1's segment loads and dup bands are emitted inside
            # chunk ci's supergroup loop so every engine queue interleaves
            # producer work for the next chunk with consumer work for the
            # current one. PSUM->staging drain alternates ACT/DVE so
            # neither engine paces PE.
            # chunks 0-1: fine granularity (4 segs/bands) to shrink the
            # pipeline fill; later chunks: coarse (2 segs/bands) to halve
            # the sync-instruction count on the critical path
            SEGS_F = [0, 10, 18, 26, CR + 2]
            SEGS_C = [0, 18, CR + 2]
            BANDS_F = [0, 8, 16, 24, 32]
            BANDS_C = [0, 16, 32]

            def nseg(cj):
                return 4 if cj < 2 else 2

            def emit_loads(cj):
                l0 = 1 if cj == 0 else 2
                l1 = CR + 1 if cj == NCH - 1 else CR + 2
                R0 = cj * CR
                segs = SEGS_F if cj < 2 else SEGS_C
                for si in range(nseg(cj)):
                    a = max(segs[si], l0)
                    b = min(segs[si + 1], l1)
                    nc.sync.dma_start(
                        out=xxb[cj % 4][0:CIN, a:b, 1:W + 1],
                        in_=xt[:, R0 - 1 + a:R0 - 1 + b, :])

            def emit_band(cj, si):
                xx = xxb[cj % 4]
                if si == 0:
                    if cj == 0:
                        nc.vector.tensor_copy(xx[0:CIN, 0:1, :], zrow)
                    else:
                        nc.vector.tensor_copy(
                            xx[0:CIN, 0:2, :],
                            xxb[(cj - 1) % 4][0:CIN, CR:CR + 2, :])
                bands = BANDS_F if cj < 2 else BANDS_C
                if si == nseg(cj) - 1 and cj == NCH - 1:
                    nc.vector.tensor_copy(xx[0:CIN, CR + 1:CR + 2, :], zrow)
                ba, bb = bands[si], bands[si + 1]
                nc.vector.tensor_copy(xx[CIN:2 * CIN, ba:bb, :],
                                      xx[0:CIN, ba + 2:bb + 2, :])

            wstd_emitted = False
            for _ in range(repeats):
                emit_loads(0)
                emit_loads(1)
                for si in range(4):
                    emit_band(0, si)
                for ci in range(NCH):
                    xx = xxb[ci % 4]
                    xxr = xx.rearrange("p (a b) w -> p b a w", b=2)
                    for q in range(SGC):
                        if q == 0 and ci + 2 < NCH:
                            emit_loads(ci + 2)
                        if ci + 1 < NCH and q % 2 == 1:
                            si = (q - 1) // 2
                            if si < nseg(ci + 1):
                                emit_band(ci + 1, si)
                        sg = ci * SGC + q
                        k2 = sg % 2
                        if k2 == 0:
                            ys = yout.tile([2 * NF, 2 * 2 * W], f16)
                        ps = accp.tile([2 * NF, 2 * W], f32)
                        for s in range(KK):
                            nc.tensor.matmul(
                                ps, lhsT=L1[:, s, :],
                                rhs=xxr[:, 0, 2 * q:2 * q + 2, s:s + W],
                                start=(s == 0), stop=False)
                        for s in range(KK):
                            nc.tensor.matmul(
                                ps, lhsT=L2[:, s, :],
                                rhs=xxr[:, 1, 2 * q:2 * q + 2, s:s + W],
                                start=False, stop=(s == KK - 1))
                        if not wstd_emitted:
                            emit_wstd()
                            wstd_emitted = True
                        yslice = ys[:, k2 * 2 * W:(k2 + 1) * 2 * W]
                        # drain applies demod scale + bias:
                        # y = ps*wstd[n] + cb[n]
                        if q % 2 == 0:
                            nc.scalar.activation(
                                yslice, ps,
                                mybir.ActivationFunctionType.Identity,
                                bias=cb2_sb, scale=wstd2)
                        else:
                            nc.vector.tensor_scalar(
                                yslice, ps, wstd2, cb2_sb,
                                op0=mybir.AluOpType.mult,
                                op1=mybir.AluOpType.add)
                        if k2 == 1:
                            nc.scalar.dma_start(
                                out=ytd[:, (sg - 1) * 2 * W:(sg + 1) * 2 * W],
                                in_=ys)

    nc.compile()
    return nc


def _get(repeats=1):
    if repeats not in _CACHE:
        _CACHE[repeats] = _build(repeats)
    return _CACHE[repeats]


def kernel(x, w, fce_kernel, fce_bias, conv_kernel, conv_bias):
    from concourse.bass_utils import run_bass_kernel_spmd

    nc = _get()
    in_maps = []
    for b in range(B):
        in_maps.append({
            "xt": np.ascontiguousarray(
                np.asarray(x[b], np.float32).transpose(2, 0, 1)).astype(np.float16),
            "wv": np.ascontiguousarray(np.asarray(w[b], np.float32)),
            "fce_k": np.asarray(fce_kernel, np.float32),
            "fce_b": np.asarray(fce_bias, np.float32),
            "ck": np.asarray(conv_kernel, np.float32).astype(np.float16),
            "cb": np.asarray(conv_bias, np.float32),
        })
    res = run_bass_kernel_spmd(nc, in_maps, core_ids=list(range(NCORES)))
    out = np.empty((B, H, W, NF), np.float32)
    for b in range(B):
        a = np.asarray(res.results[b]["ytd"]).astype(np.float32)
        # [ro*64+n, g*W+col] -> [h, w, n] with h = 2g + ro
        a = a.reshape(2, NF, H // 2, W).transpose(2, 0, 3, 1)
        out[b] = a.reshape(H, W, NF)
    return out


# revision 8
# speedup vs baseline: 1.1912x; 1.0236x over previous
"""ConvMod kernel for Trainium2 (8 NeuronCores, batch-parallel).

Per-sample modulated 3x3 grouped conv:
  style = w @ (fce_kernel*fce_scale) + fce_bias                [B, CIN]
  wp    = conv_kernel * conv_scale * style                     [B,3,3,CIN,NF]
  wpp   = wp * rsqrt(sum(wp^2, (ky,kx,cin)) + 1e-8)            demodulated
  out   = conv2d_same(x, wpp per-sample) + conv_bias           [B,H,W,NF]

Sharding: batch B=8 across 8 cores (1 sample/core), params replicated.

Device layout: M=128 matmul packing. PSUM partitions hold (2 output rows x
64 channels). The x tile duplicates channels on partitions 64-127 shifted
by +2 rows, so one K=128 matmul contracts two input rows at once with a
block-structured lhsT. Per 4 output rows (supergroup): 6 matmuls of
free-size 512 cover all 9 taps for all 4 rows.

Key scheduling facts this file is built around (TimelineSim cost model):
 - demod scales only the output channel, so conv(x, wp*diag(wstd)) =
   conv(x, wp)*wstd[n]: wstd is applied as a per-partition scale in the
   PSUM->SBUF drain (fused with the conv_bias add), keeping the sqrt
   chain off the critical path. ssum[n] = sum_c stylec[c]^2 *
   (sum_t ck[c,t,n]^2) collapses the reduction to one tiny matmul whose
   [64,1] result lands in drain-scale layout directly.
 - the PE p-state ramp needs ~3us of continuous execution to reach
   2.4GHz and resets on multi-us idle gaps; dummy warm-up matmuls keep
   PE busy from t~0.5us until the style chain's inputs land, so the
   conv runs at full clock from its first instruction.
 - per-DMA fixed costs (SEQ 565/HWDGE 625/delay 650/sem 900ns) dominate
   small transfers: weights are host-repacked to contiguous f16 so all
   pre-x traffic is ~0.4us, and wv/fce_b/cb ride the Pool SWDGE
   generator, which runs in parallel with the HWDGE stream.
 - x and all conv weights travel as f16 (tolerance 2e-2 >> f16
   rounding): halves input DMA and on-chip dup-copy cost; f16 matmul
   rate equals f32r on TRN2 (1 col/cycle).
"""

import numpy as np

B, H, W, CIN = 8, 256, 256, 64
WDIM, NF, KK = 512, 64, 3
NCORES = 8
CR = 32                 # output rows per x chunk
NCH = H // CR           # 8 chunks
SGC = CR // 4           # supergroups (4 output rows) per chunk
JW = WDIM // 128
FCE_SCALE = float(np.sqrt(1.0 / WDIM))
CONV_SCALE = float(np.sqrt(1.0 / 0.6 / (KK * KK * CIN)))
NWARM = 30

_CACHE = {}


def _build(repeats=1):
    import concourse.mybir as mybir
    import concourse.tile as tile
    from concourse import bacc

    f32 = mybir.dt.float32
    f16 = mybir.dt.float16
    nc = bacc.Bacc("TRN2", target_bir_lowering=False, debug=False,
                   num_devices=NCORES)

    # all weight tensors host-repacked into their SBUF layouts, f16
    xt = nc.dram_tensor("xt", [CIN, H, W], f16, kind="ExternalInput").ap()
    wv = nc.dram_tensor("wv", [128, JW], f16, kind="ExternalInput").ap()
    fce_k = nc.dram_tensor("fce_k", [128, JW, CIN], f16,
                           kind="ExternalInput").ap()
    fce_b = nc.dram_tensor("fce_b", [CIN], f32, kind="ExternalInput").ap()
    ck_d = nc.dram_tensor("ck", [CIN, KK * KK, NF], f16,
                          kind="ExternalInput").ap()
    cb_d = nc.dram_tensor("cb", [NF], f32, kind="ExternalInput").ap()
    # out: partition p = ro*64 + n (ro = row parity), free = (g, col) with
    # output row = 2g + ro
    ytd = nc.dram_tensor("ytd", [2 * NF, (H // 2) * W], f16,
                         kind="ExternalOutput").ap()

    NT = KK * KK  # 9 taps
    # chunk 0 starts extra fine so the first supergroup's band is ready
    # early; chunk 1 fine; later chunks coarse (fewer sync instructions)
    SEGS = {0: [0, 6, 10, 18, 26, CR + 2], 1: [0, 10, 18, 26, CR + 2]}
    BANDS = {0: [0, 4, 8, 16, 24, CR], 1: [0, 8, 16, 24, CR]}
    SEGS_C = [0, 18, CR + 2]
    BANDS_C = [0, 16, CR]

    def nseg(cj):
        return len(SEGS.get(cj, SEGS_C)) - 1

    with tile.TileContext(nc) as tc:
        with (
            tc.tile_pool(name="const", bufs=1) as const,
            tc.tile_pool(name="prep", bufs=1) as prep,
            tc.tile_pool(name="pps", bufs=1, space="PSUM") as pps,
            tc.tile_pool(name="xin", bufs=1) as xin,
            tc.tile_pool(name="yout", bufs=4) as yout,
            tc.tile_pool(name="acc", bufs=7, space="PSUM") as accp,
        ):
            # dummy Sqrt first on ACT: hoists the activation-table load
            dmy = const.tile([1, 1], f32)
            nc.vector.memset(dmy, 1.0)
            dmy2 = prep.tile([1, 1], f32)
            nc.scalar.sqrt(dmy2, dmy)

            # warm-up operands via Pool (its queue is otherwise idle now)
            wrm_l = const.tile([1, 1], f16)
            nc.gpsimd.memset(wrm_l, 0.0)
            wrm_r = const.tile([1, 128], f16)
            nc.gpsimd.memset(wrm_r, 0.0)

            # SP/HWDGE: ck first (gates L build + ckk chain), then fce;
            # x segments follow. Pool/SWDGE: wv, fce_b, conv_bias.
            ck_sb = prep.tile([CIN, NT, NF], f16)
            nc.sync.dma_start(out=ck_sb, in_=ck_d)
            fce_sb = prep.tile([128, JW, CIN], f16)
            nc.sync.dma_start(out=fce_sb, in_=fce_k)
            wv_sb = prep.tile([128, JW], f16)
            nc.gpsimd.dma_start(out=wv_sb, in_=wv)
            fce_b_sb = prep.tile([CIN, 1], f32)
            nc.gpsimd.dma_start(out=fce_b_sb, in_=fce_b)
            cb2_sb = const.tile([2 * NF, 1], f32)
            nc.gpsimd.dma_start(out=cb2_sb[0:NF, :], in_=cb_d)
            nc.gpsimd.dma_start(out=cb2_sb[NF:2 * NF, :], in_=cb_d)

            # zero row for padding writes
            zrow = const.tile([CIN, 1, W + 2], f16)
            nc.vector.memset(zrow.rearrange("c a w -> c (a w)"), 0.0)

            # persistent x tiles: A half = x rows R0-1..R0+CR (CR+2),
            # B half (partitions 64-127) = A shifted +2 rows. Col 0 and
            # W+1 of the A half are zero borders (B inherits via the copy).
            xxb = [xin.tile([2 * CIN, CR + 2, W + 2], f16, name=f"xx{k}")
                   for k in range(4)]
            zcol = zrow[:, 0:1, 0:CR + 2].rearrange("c a w -> c w a")
            for k in range(4):
                nc.vector.tensor_copy(xxb[k][0:CIN, :, 0:1], zcol)
                nc.vector.tensor_copy(xxb[k][0:CIN, :, W + 1:W + 2], zcol)

            # PE warm-up matmuls, then the style chain
            ppsbig = pps.tile([128, 512], f32)
            wrm_ps = ppsbig[0:1, 384:512]
            for _w in range(NWARM):
                nc.tensor.matmul(wrm_ps, lhsT=wrm_l, rhs=wrm_r,
                                 start=True, stop=True)
            style_ps = ppsbig[0:CIN, 0:1]
            for j in range(JW):
                nc.tensor.matmul(style_ps, lhsT=fce_sb[:, j, :],
                                 rhs=wv_sb[:, j:j + 1],
                                 start=(j == 0), stop=(j == JW - 1))
            fce_b_sc = prep.tile([CIN, 1], f32)
            nc.scalar.mul(out=fce_b_sc, in_=fce_b_sb, mul=CONV_SCALE)
            stylec = prep.tile([CIN, 1], f32)
            nc.scalar.activation(stylec, style_ps,
                                 mybir.ActivationFunctionType.Identity,
                                 bias=fce_b_sc, scale=FCE_SCALE * CONV_SCALE)

            # ckk[c,n] = sum_t ck^2 on DVE (ck-gated, runs while stylec
            # is still pending)
            sq = prep.tile([CIN, NT, NF], f32)
            nc.vector.tensor_mul(sq.rearrange("c t n -> c (t n)"),
                                 ck_sb.rearrange("c t n -> c (t n)"),
                                 ck_sb.rearrange("c t n -> c (t n)"))
            t4 = prep.tile([CIN, 4, NF], f32)
            nc.vector.tensor_add(t4.rearrange("c t n -> c (t n)"),
                                 sq[:, 0:4, :].rearrange("c t n -> c (t n)"),
                                 sq[:, 4:8, :].rearrange("c t n -> c (t n)"))
            t2 = prep.tile([CIN, 2, NF], f32)
            nc.vector.tensor_add(t2.rearrange("c t n -> c (t n)"),
                                 t4[:, 0:2, :].rearrange("c t n -> c (t n)"),
                                 t4[:, 2:4, :].rearrange("c t n -> c (t n)"))
            ckk = prep.tile([CIN, NF], f32)
            nc.vector.tensor_add(ckk, t2[:, 0, :], t2[:, 1, :])
            nc.vector.tensor_add(ckk, ckk, sq[:, 8, :])
            stylec2 = prep.tile([CIN, 1], f32)
            nc.vector.tensor_mul(stylec2, stylec, stylec)
            eps_sb = prep.tile([CIN, 1], f32)
            nc.vector.memset(eps_sb, 1e-8)

            # Block-structured lhsT tiles, straight from ck * stylec (no
            # demod mul -- applied at drain time).
            # lhsT[k, m]: k<64 = channels of x row XA, k>=64 = x row XA+2;
            # m<64 = out row r (ro=0) channels, m>=64 = out row r+1 (ro=1).
            # mm1 (XA = r-1): (A,ro0)=w[-1,s-1] (B,ro0)=w[+1,s-1]
            #                 (B,ro1)=w[0,s-1]  (A,ro1)=0
            # mm2 (XA = r):   (A,ro0)=w[0,s-1]  (A,ro1)=w[-1,s-1]
            #                 (B,ro1)=w[+1,s-1] (B,ro0)=0
            # wp tap index t = (dy+1)*3 + (dx+1), dx = s-1.
            L1 = const.tile([2 * CIN, KK, 2 * NF], f16)
            L2 = const.tile([2 * CIN, KK, 2 * NF], f16)
            nc.gpsimd.memset(L1[0:CIN, :, NF:2 * NF], 0.0)
            nc.gpsimd.memset(L2[CIN:2 * CIN, :, 0:NF], 0.0)

            def lmul(dst, t0):
                nc.vector.tensor_scalar_mul(dst, ck_sb[:, t0:t0 + 3, :],
                                            stylec)

            lmul(L1[0:CIN, :, 0:NF], 0)
            lmul(L1[CIN:2 * CIN, :, 0:NF], 6)
            lmul(L1[CIN:2 * CIN, :, NF:2 * NF], 3)
            lmul(L2[0:CIN, :, 0:NF], 3)
            lmul(L2[0:CIN, :, NF:2 * NF], 0)
            lmul(L2[CIN:2 * CIN, :, NF:2 * NF], 6)

            # ssum right after the style matmuls in the PE stream (its
            # inputs are ready before the first conv matmul, so nothing
            # behind it blocks); sroot on ACT before the first drain.
            ssum_ps = ppsbig[0:CIN, 64:65]
            nc.tensor.matmul(ssum_ps, lhsT=ckk, rhs=stylec2,
                             start=True, stop=True)
            sroot = prep.tile([CIN, 1], f32)
            nc.scalar.activation(sroot, ssum_ps,
                                 mybir.ActivationFunctionType.Sqrt,
                                 bias=eps_sb, scale=1.0)

            def emit_loads(cj):
                l0 = 1 if cj == 0 else 2
                l1 = CR + 1 if cj == NCH - 1 else CR + 2
                R0 = cj * CR
                segs = SEGS.get(cj, SEGS_C)
                for si in range(nseg(cj)):
                    a = max(segs[si], l0)
                    b = min(segs[si + 1], l1)
                    nc.sync.dma_start(
                        out=xxb[cj % 4][0:CIN, a:b, 1:W + 1],
                        in_=xt[:, R0 - 1 + a:R0 - 1 + b, :])

            def emit_band(cj, si):
                xx = xxb[cj % 4]
                if si == 0:
                    if cj == 0:
                        nc.vector.tensor_copy(xx[0:CIN, 0:1, :], zrow)
                    else:
                        nc.vector.tensor_copy(
                            xx[0:CIN, 0:2, :],
                            xxb[(cj - 1) % 4][0:CIN, CR:CR + 2, :])
                bands = BANDS.get(cj, BANDS_C)
                if si == nseg(cj) - 1 and cj == NCH - 1:
                    nc.vector.tensor_copy(xx[0:CIN, CR + 1:CR + 2, :], zrow)
                ba, bb = bands[si], bands[si + 1]
                nc.vector.tensor_copy(xx[CIN:2 * CIN, ba:bb, :],
                                      xx[0:CIN, ba + 2:bb + 2, :])

            def emit_fill():
                emit_loads(0)
                emit_loads(1)
                for si in range(nseg(0)):
                    emit_band(0, si)

            # chunk 0/1 loads + chunk 0 bands BEFORE the wstd tail ops so
            # the scheduler cannot park the (late-ready) reciprocal ahead
            # of them in the DVE stream.
            emit_fill()

            wstdT = prep.tile([CIN, 1], f32)
            nc.vector.reciprocal(wstdT, sroot)
            wstd2 = const.tile([2 * NF, 1], f32)
            nc.vector.tensor_copy(wstd2[0:NF, :], wstdT)
            nc.vector.tensor_copy(wstd2[NF:2 * NF, :], wstdT)

            # ---- main conv loop (software-pipelined emission) ----
            # Chunk ci+1's segment loads and dup bands are emitted inside
            # chunk ci's supergroup loop so every engine queue interleaves
            # producer work for the next chunk with consumer work for the
            # current one. PSUM->staging drain alternates ACT/DVE so
            # neither engine paces PE; drains fuse the demod scale and
            # bias: y = ps*wstd[n] + cb[n].
            first = True
            for _ in range(repeats):
                if not first:
                    emit_fill()
                first = False
                for ci in range(NCH):
                    xx = xxb[ci % 4]
                    xxr = xx.rearrange("p (a b) w -> p b a w", b=2)
                    for q in range(SGC):
                        if q == 0 and ci + 2 < NCH:
                            emit_loads(ci + 2)
                        if ci + 1 < NCH and q % 2 == 1:
                            si = (q - 1) // 2
                            if si < nseg(ci + 1):
                                emit_band(ci + 1, si)
                        sg = ci * SGC + q
                        k2 = sg % 2
                        if k2 == 0:
                            ys = yout.tile([2 * NF, 2 * 2 * W], f16)
                        ps = accp.tile([2 * NF, 2 * W], f32)
                        for s in range(KK):
                            nc.tensor.matmul(
                                ps, lhsT=L1[:, s, :],
                                rhs=xxr[:, 0, 2 * q:2 * q + 2, s:s + W],
                                start=(s == 0), stop=False)
                        for s in range(KK):
                            nc.tensor.matmul(
                                ps, lhsT=L2[:, s, :],
                                rhs=xxr[:, 1, 2 * q:2 * q + 2, s:s + W],
                                start=False, stop=(s == KK - 1))
                        yslice = ys[:, k2 * 2 * W:(k2 + 1) * 2 * W]
                        if q % 2 == 0:
                            nc.scalar.activation(
                                yslice, ps,
                                mybir.ActivationFunctionType.Identity,
                                bias=cb2_sb, scale=wstd2)
                        else:
                            nc.vector.tensor_scalar(
                                yslice, ps, wstd2, cb2_sb,
                                op0=mybir.AluOpType.mult,
                                op1=mybir.AluOpType.add)
                        if k2 == 1:
                            nc.scalar.dma_start(
                                out=ytd[:, (sg - 1) * 2 * W:(sg + 1) * 2 * W],
                                in_=ys)

    nc.compile()
    return nc


def _get(repeats=1):
    if repeats not in _CACHE:
        _CACHE[repeats] = _build(repeats)
    return _CACHE[repeats]


def _pack(x_b, w_b, fce_kernel, fce_bias, conv_kernel, conv_bias):
    f16 = np.float16
    return {
        "xt": np.ascontiguousarray(
            np.asarray(x_b, np.float32).transpose(2, 0, 1)).astype(f16),
        "wv": np.ascontiguousarray(
            np.asarray(w_b, np.float32).reshape(JW, 128).T).astype(f16),
        "fce_k": np.ascontiguousarray(
            np.asarray(fce_kernel, np.float32)
            .reshape(JW, 128, CIN).transpose(1, 0, 2)).astype(f16),
        "fce_b": np.asarray(fce_bias, np.float32),
        "ck": np.ascontiguousarray(
            np.asarray(conv_kernel, np.float32)
            .transpose(2, 0, 1, 3).reshape(CIN, KK * KK, NF)).astype(f16),
        "cb": np.asarray(conv_bias, np.float32),
    }


def kernel(x, w, fce_kernel, fce_bias, conv_kernel, conv_bias):
    from concourse.bass_utils import run_bass_kernel_spmd

    nc = _get()
    in_maps = [_pack(x[b], w[b], fce_kernel, fce_bias,
                     conv_kernel, conv_bias) for b in range(B)]
    res = run_bass_kernel_spmd(nc, in_maps, core_ids=list(range(NCORES)))
    out = np.empty((B, H, W, NF), np.float32)
    for b in range(B):
        a = np.asarray(res.results[b]["ytd"]).astype(np.float32)
        # [ro*64+n, g*W+col] -> [h, w, n] with h = 2g + ro
        a = a.reshape(2, NF, H // 2, W).transpose(2, 0, 3, 1)
        out[b] = a.reshape(H, W, NF)
    return out
